# revision 36
# baseline (speedup 1.0000x reference)
"""Trainium2 Bass kernel for 2-layer GAT (nn_GAT_50603304681766).

Strategy: partition nodes (destinations) across 8 cores. Each core:
  t1 = x_shard @ [W1 | W1@Asrc | W1@Adst]  (PE, fp8 x, fp16 weights)
  -> pack [h|s_hi|s_lo] bf16 rows -> AllGather table T1
  per dst-tile (128 nodes): gather T1[src] rows via indirect DMA,
  build one-hot scatter matrix M and its transpose MT on device
  (iota + is_equal against the per-edge dst-slot array in both
  layouts; MT uses a partition-broadcast DMA), d-expand via MT@dtab,
  g = exp(leakyrelu(s+d)), weighted scatter matmul into PSUM
  (messages + denominator), normalize, +bias, ELU, then fused
  t2 = h2 @ W2a packs table T2 -> AllGather -> layer-2 message pass
  -> log_softmax -> uint8-quantized output.
Only compact edge indices (uint16 src rows + bf16 dst slots) are
shipped to the device; the big one-hot matrices are built on-chip.
The host<->device tunnel here moves ~50 MB/s, so all transfers are
minimized: x fp8, output uint8 ([-12,0] range), weights fp16.
"""
import numpy as np
import ml_dtypes

N = 50000
E0 = 800000
F_IN = 256
H = 4
C1 = 64
C2 = 32
NEG = 0.2
NC = 8
NSH = 6250            # dst nodes per core
NSHP = 6272           # padded to 49*128
NT = 49               # dst tiles per core
NBLK = 18             # edge blocks (of 128) per dst tile
ROWS = NC * NSHP      # allgathered table rows = 50176
RW1 = 264             # T1 row: h(256) + s_hi(4) + s_lo(4)  [bf16]
RW2 = 136             # T2 row: h2'(128) + s2_hi(4) + s2_lo(4) [bf16]

bf = ml_dtypes.bfloat16


def _host_prep(x, edge_index, W1, as1, ad1, b1, W2, as2, ad2, b2):
    ei = np.asarray(edge_index)
    src = np.concatenate([ei[0], np.arange(N, dtype=ei.dtype)]).astype(np.int64)
    dst = np.concatenate([ei[1], np.arange(N, dtype=ei.dtype)]).astype(np.int64)
    ET = src.shape[0]

    # augmented weights: t = x @ [W | W@S | W@D]; s/d per head
    def aug(W, a_s, a_d, heads, ch):
        S = np.zeros((heads * ch, heads), np.float32)
        D = np.zeros((heads * ch, heads), np.float32)
        for h in range(heads):
            S[h * ch:(h + 1) * ch, h] = a_s[h]
            D[h * ch:(h + 1) * ch, h] = a_d[h]
        return np.concatenate([W, W @ S, W @ D], axis=1)  # [fin, hc+2h]

    W1a = aug(np.asarray(W1, np.float32), np.asarray(as1), np.asarray(ad1),
              H, C1).astype(np.float16)                   # [256, 264]
    W2a = aug(np.asarray(W2, np.float32), np.asarray(as2), np.asarray(ad2),
              H, C2).astype(np.float16)                   # [256, 136]

    core_of = dst // NSH
    loc = dst - core_of * NSH
    gtile = core_of * NT + loc // 128   # global dst tile id, 0..NC*NT
    dloc = loc % 128
    srow = ((src // NSH) * NSHP + (src % NSH)).astype(np.int32)

    order = np.argsort(gtile, kind="stable")
    counts = np.bincount(gtile, minlength=NC * NT)
    assert counts.max() <= NBLK * 128, f"tile overflow {counts.max()}"
    starts = np.zeros(NC * NT, np.int64)
    starts[1:] = np.cumsum(counts)[:-1]
    gs = gtile[order]
    pos = np.arange(ET, dtype=np.int64) - starts[gs]
    idx_flat = np.zeros((NC * NT, NBLK * 128), np.int32)
    idx_flat[gs, pos] = srow[order]
    dl_flat = np.full((NC * NT, NBLK * 128), 255, np.int32)
    dl_flat[gs, pos] = dloc[order]
    # per-block layout: lane p of block b <- flat[b*128 + p]
    idx_t = np.ascontiguousarray(
        idx_flat.reshape(NC, NT, NBLK, 128).transpose(0, 1, 3, 2)).astype(np.uint16)
    dl_t = np.ascontiguousarray(
        dl_flat.reshape(NC, NT, NBLK, 128).transpose(0, 1, 3, 2)).astype(bf)
    dlT_t = dl_flat.reshape(NC, NT, NBLK, 128).astype(bf)

    f8 = ml_dtypes.float8_e4m3fn
    xs = np.zeros((NC, F_IN, NSHP), f8)
    xf = np.asarray(x, np.float32).astype(f8)
    for c in range(NC):
        xs[c, :, :NSH] = xf[c * NSH:(c + 1) * NSH].T

    b1r = np.asarray(b1, np.float32)[None, :]
    b2r = np.asarray(b2, np.float32)[None, :]
    return W1a, W2a, idx_t, dl_t, dlT_t, xs, b1r, b2r


def _build_nc():
    import concourse.bass as bass
    import concourse.tile as tile
    from concourse import mybir
    from concourse.bass import IndirectOffsetOnAxis

    f32 = mybir.dt.float32
    f16 = mybir.dt.float16
    f8 = mybir.dt.float8e4
    bf16 = mybir.dt.bfloat16
    i32 = mybir.dt.int32
    u16 = mybir.dt.uint16
    AF = mybir.ActivationFunctionType
    ALU = mybir.AluOpType

    nc = bass.Bass()
    xT = nc.declare_dram_parameter("xT", [F_IN, NSHP], f8, isOutput=False)
    w1 = nc.declare_dram_parameter("w1", [F_IN, RW1], f16, isOutput=False)
    w2 = nc.declare_dram_parameter("w2", [F_IN, RW2], f16, isOutput=False)
    idxp = nc.declare_dram_parameter("idx", [NT, 128, NBLK], u16, isOutput=False)
    dlp = nc.declare_dram_parameter("dl", [NT, 128, NBLK], bf16, isOutput=False)
    dlTp = nc.declare_dram_parameter("dlT", [NT, NBLK, 128], bf16, isOutput=False)
    b1p = nc.declare_dram_parameter("b1r", [1, H * C1], f32, isOutput=False)
    b2p = nc.declare_dram_parameter("b2r", [1, H * C2], f32, isOutput=False)
    outp = nc.declare_dram_parameter("out", [NT, 128, H * C2], mybir.dt.uint8,
                                     isOutput=True)

    t1_loc = nc.dram_tensor("t1_loc", [NSHP, RW1], bf16)
    d1_loc = nc.dram_tensor("d1_loc", [NSHP, 8], bf16)
    t2_loc = nc.dram_tensor("t2_loc", [NSHP, RW2], bf16)
    d2_loc = nc.dram_tensor("d2_loc", [NSHP, 8], bf16)
    T1 = nc.dram_tensor("T1ag", [ROWS, RW1], bf16, addr_space="Shared")
    T2 = nc.dram_tensor("T2ag", [ROWS, RW2], bf16, addr_space="Shared")

    # ---------- phase 1: t1 = xT.T @ W1a ; pack tables ----------
    with tile.TileContext(nc) as tc:
        with (
            tc.tile_pool(name="w", bufs=1) as wp,
            tc.tile_pool(name="a", bufs=3) as ap,
            tc.tile_pool(name="ps", bufs=2, space="PSUM") as pp,
        ):
            w1_t = wp.tile([128, 2, RW1], f16)
            nc.sync.dma_start(w1_t[:], w1[:, :].rearrange("(k p) c -> p k c", p=128))
            for t in range(NT):
                xt8 = ap.tile([128, 2, 128], f8, tag="xt8")
                nc.sync.dma_start(
                    xt8[:],
                    xT[:, t * 128:(t + 1) * 128].rearrange("(k p) c -> p k c", p=128))
                xt = ap.tile([128, 2, 128], f16, tag="xt")
                nc.vector.tensor_copy(xt[:], xt8[:])
                acc = pp.tile([128, RW1], f32, tag="acc")
                nc.tensor.matmul(out=acc[:], lhsT=xt[:, 0, :],
                                 rhs=w1_t[:, 0, :], start=True, stop=False)
                nc.tensor.matmul(out=acc[:], lhsT=xt[:, 1, :],
                                 rhs=w1_t[:, 1, :], start=False, stop=True)
                # pack row: h bf16, s hi/lo
                row = ap.tile([128, RW1], bf16, tag="row")
                nc.vector.tensor_copy(row[:, 0:256], acc[:, 0:256])
                s_hi32 = ap.tile([128, 4], f32, tag="shi32")
                nc.vector.tensor_copy(row[:, 256:260], acc[:, 256:260])
                nc.vector.tensor_copy(s_hi32[:], row[:, 256:260])
                s_lo = ap.tile([128, 4], f32, tag="slo")
                nc.vector.tensor_tensor(out=s_lo[:], in0=acc[:, 256:260],
                                        in1=s_hi32[:], op=ALU.subtract)
                nc.vector.tensor_copy(row[:, 260:264], s_lo[:])
                nc.sync.dma_start(t1_loc[t * 128:(t + 1) * 128, :], row[:])
                # d table hi/lo
                drow = ap.tile([128, 8], bf16, tag="drow")
                d_hi32 = ap.tile([128, 4], f32, tag="dhi32")
                nc.vector.tensor_copy(drow[:, 0:4], acc[:, 260:264])
                nc.vector.tensor_copy(d_hi32[:], drow[:, 0:4])
                d_lo = ap.tile([128, 4], f32, tag="dlo")
                nc.vector.tensor_tensor(out=d_lo[:], in0=acc[:, 260:264],
                                        in1=d_hi32[:], op=ALU.subtract)
                nc.vector.tensor_copy(drow[:, 4:8], d_lo[:])
                nc.sync.dma_start(d1_loc[t * 128:(t + 1) * 128, :], drow[:])

    with nc.semaphore("cc1") as cc1:
        nc.gpsimd.collective_compute(
            "AllGather", mybir.AluOpType.bypass,
            replica_groups=[list(range(NC))],
            ins=[t1_loc[:, :].opt()], outs=[T1[:, :].opt()],
        ).then_inc(cc1, 1)
        nc.gpsimd.wait_ge(cc1, 1)

    # ---------- phase 2: L1 message passing -> h2, pack T2 ----------
    def message_pass(tc, Tag, d_loc_t, rw, hw, out_cb):
        """hw = feature width (256 / 128); rw = table row width."""
        from concourse import mybir
        ALU = mybir.AluOpType
        AF = mybir.ActivationFunctionType
        with (
            tc.tile_pool(name="mp_c", bufs=1) as cp,
            tc.tile_pool(name="mp_v", bufs=3) as vp,
            tc.tile_pool(name="mp_m", bufs=2) as mp_,
            tc.tile_pool(name="mp_s", bufs=2) as sp,
            tc.tile_pool(name="mp_ps", bufs=2, space="PSUM") as pp,
            tc.tile_pool(name="mp_ps2", bufs=2, space="PSUM") as pp2,
        ):
            # constants: row-iota (f32) and per-partition iota (f32)
            eqi = cp.tile([128, 128], mybir.dt.float32)
            i2 = cp.tile([128, 128], mybir.dt.int32)
            nc.gpsimd.iota(i2[:], pattern=[[1, 128]], base=0,
                           channel_multiplier=0)
            nc.vector.tensor_copy(eqi[:], i2[:])
            iotp = cp.tile([128, 1], mybir.dt.int32)
            nc.gpsimd.iota(iotp[:], pattern=[[0, 1]], base=0,
                           channel_multiplier=1)
            iotf = cp.tile([128, 1], mybir.dt.float32)
            nc.vector.tensor_copy(iotf[:], iotp[:])
            for t in range(NT):
                idx_u = sp.tile([128, NBLK], mybir.dt.uint16, tag="idxu")
                nc.sync.dma_start(idx_u[:], idxp[t, :, :])
                idx_t = sp.tile([128, NBLK], mybir.dt.int32, tag="idx")
                nc.vector.tensor_copy(idx_t[:], idx_u[:])
                dl_t = sp.tile([128, NBLK], mybir.dt.bfloat16, tag="dl")
                nc.sync.dma_start(dl_t[:], dlp[t, :, :])
                dlf = sp.tile([128, NBLK], mybir.dt.float32, tag="dlf")
                nc.vector.tensor_copy(dlf[:], dl_t[:])
                dtab = sp.tile([128, 8], mybir.dt.bfloat16, tag="dtab")
                nc.sync.dma_start(dtab[:], d_loc_t[t * 128:(t + 1) * 128, :])
                v = vp.tile([128, NBLK, rw], mybir.dt.bfloat16, tag="v")
                for b in range(NBLK):
                    nc.gpsimd.indirect_dma_start(
                        out=v[:, b, :], out_offset=None, in_=Tag[:, :],
                        in_offset=IndirectOffsetOnAxis(ap=idx_t[:, b:b + 1], axis=0))
                # build one-hot M on device: M[e, (b,d)] = (dl[e,b] == d)
                m_t = mp_.tile([128, NBLK, 128], mybir.dt.bfloat16, tag="m")
                nc.vector.tensor_tensor(
                    out=m_t[:],
                    in0=dlf[:].unsqueeze(2).to_broadcast([128, NBLK, 128]),
                    in1=eqi[:].unsqueeze(1).to_broadcast([128, NBLK, 128]),
                    op=ALU.is_equal)
                # MT[d, (b,e)] = (dlT[b,e] == d): partition-broadcast DMA + cmp
                dlT_b = mp_.tile([128, NBLK, 128], mybir.dt.bfloat16, tag="dlTb")
                nc.sync.dma_start(
                    dlT_b[:],
                    dlTp[t, :, :].unsqueeze(0).to_broadcast([128, NBLK, 128]))
                mt_t = mp_.tile([128, NBLK, 128], mybir.dt.bfloat16, tag="mt")
                nc.vector.tensor_scalar(out=mt_t[:], in0=dlT_b[:],
                                        scalar1=iotf[:, 0:1], scalar2=None,
                                        op0=ALU.is_equal)
                # d-expand dex = MT @ dtab
                dex = pp2.tile([128, NBLK * 8], mybir.dt.float32, tag="dex")
                for b in range(NBLK):
                    nc.tensor.matmul(out=dex[:, b * 8:(b + 1) * 8],
                                     lhsT=mt_t[:, b, :], rhs=dtab[:],
                                     start=True, stop=True)
                # e = s + d (hi+lo), lrelu, exp
                s32 = sp.tile([128, NBLK, 4], mybir.dt.float32, tag="s32")
                nc.vector.tensor_tensor(out=s32[:], in0=v[:, :, hw:hw + 4],
                                        in1=v[:, :, hw + 4:hw + 8], op=ALU.add)
                dsb = sp.tile([128, NBLK, 8], mybir.dt.float32, tag="dsb")
                nc.vector.tensor_copy(dsb[:], dex[:].rearrange("p (b k) -> p b k", k=8))
                d32 = sp.tile([128, NBLK, 4], mybir.dt.float32, tag="d32")
                nc.vector.tensor_tensor(out=d32[:], in0=dsb[:, :, 0:4],
                                        in1=dsb[:, :, 4:8], op=ALU.add)
                e32 = sp.tile([128, NBLK, 4], mybir.dt.float32, tag="e32")
                nc.vector.tensor_tensor(out=e32[:], in0=s32[:], in1=d32[:],
                                        op=ALU.add)
                e_s = sp.tile([128, NBLK, 4], mybir.dt.float32, tag="es")
                nc.vector.tensor_scalar_mul(e_s[:], e32[:], NEG)
                nc.vector.tensor_tensor(out=e32[:], in0=e32[:], in1=e_s[:],
                                        op=ALU.max)
                g = sp.tile([128, NBLK, 4], mybir.dt.float32, tag="g")
                nc.scalar.activation(g[:], e32[:], AF.Exp)
                # weighted rhs [hw cols scaled by g, then g cols]
                wv = vp.tile([128, NBLK, hw + 4], mybir.dt.bfloat16, tag="wv")
                nc.vector.tensor_tensor(
                    out=wv[:, :, 0:hw].rearrange("p b (h c) -> p b h c", h=4),
                    in0=v[:, :, 0:hw].rearrange("p b (h c) -> p b h c", h=4),
                    in1=g[:].unsqueeze(3).to_broadcast([128, NBLK, 4, hw // 4]),
                    op=ALU.mult)
                nc.vector.tensor_copy(wv[:, :, hw:hw + 4], g[:])
                acc = pp.tile([128, hw + 4], mybir.dt.float32, tag="acc2")
                for b in range(NBLK):
                    nc.tensor.matmul(out=acc[:], lhsT=m_t[:, b, :],
                                     rhs=wv[:, b, :], start=(b == 0),
                                     stop=(b == NBLK - 1))
                out_cb(t, acc, sp, pp2)

    with tile.TileContext(nc) as tc:
        _l1c = {}

        def l1_out(t, acc, sp, pp2):
            from concourse import mybir
            ALU = mybir.AluOpType
            AF = mybir.ActivationFunctionType
            f32 = mybir.dt.float32
            rec = sp.tile([128, 4], f32, tag="rec")
            nc.vector.reciprocal(rec[:], acc[:, 256:260])
            h2 = sp.tile([128, 256], f32, tag="h2")
            nc.vector.tensor_tensor(
                out=h2[:].rearrange("p (h c) -> p h c", h=4),
                in0=acc[:, 0:256].rearrange("p (h c) -> p h c", h=4),
                in1=rec[:].unsqueeze(2).to_broadcast([128, 4, 64]),
                op=ALU.mult)
            if "b1" not in _l1c:
                b1_t = sp.tile([128, 256], f32, tag="b1t")
                nc.sync.dma_start(b1_t[:], b1p[0:1, :].to_broadcast([128, 256]))
                _l1c["b1"] = b1_t
            nc.vector.tensor_tensor(out=h2[:], in0=h2[:], in1=_l1c["b1"][:],
                                    op=ALU.add)
            # ELU: max(x, exp(min(x,0)) - 1)
            mn = sp.tile([128, 256], f32, tag="mn")
            nc.vector.tensor_scalar_min(mn[:], h2[:], 0.0)
            nc.scalar.activation(mn[:], mn[:], AF.Exp)
            nc.vector.tensor_scalar_add(mn[:], mn[:], -1.0)
            nc.vector.tensor_tensor(out=h2[:], in0=h2[:], in1=mn[:], op=ALU.max)
            # transpose h2 -> h2T [256, 128] in psum, save to dram
            if "idn" not in _l1c:
                idn = sp.tile([128, 128], f32, tag="idn")
                iot = sp.tile([128, 1], mybir.dt.int32, tag="iot")
                nc.gpsimd.iota(iot[:], pattern=[[0, 1]], base=0,
                               channel_multiplier=1)
                iotf = sp.tile([128, 1], f32, tag="iotf")
                nc.vector.tensor_copy(iotf[:], iot[:])
                eqi = sp.tile([128, 128], f32, tag="eqi")
                i2 = sp.tile([128, 128], mybir.dt.int32, tag="i2")
                nc.gpsimd.iota(i2[:], pattern=[[1, 128]], base=0,
                               channel_multiplier=0)
                nc.vector.tensor_copy(eqi[:], i2[:])
                nc.vector.tensor_tensor(
                    out=idn[:], in0=eqi[:],
                    in1=iotf[:].to_broadcast([128, 128]), op=ALU.is_equal)
                _l1c["idn"] = idn
            idn = _l1c["idn"]
            tps = sp.tile([128, 2, 128], mybir.dt.float16, tag="tps")
            for kk in range(2):
                tp = pp2.tile([128, 128], f32, tag="tp")
                nc.tensor.transpose(out=tp[:], in_=h2[:, kk * 128:(kk + 1) * 128],
                                    identity=idn[:])
                nc.vector.tensor_copy(tps[:, kk, :], tp[:])
            # fused t2 = h2 @ W2a, pack T2 rows
            if "w2" not in _l1c:
                w2_t = sp.tile([128, 2, RW2], mybir.dt.float16, tag="w2t")
                nc.sync.dma_start(w2_t[:],
                                  w2[:, :].rearrange("(k p) c -> p k c", p=128))
                _l1c["w2"] = w2_t
            w2_t = _l1c["w2"]
            acc3 = pp2.tile([128, RW2], f32, tag="acc3")
            nc.tensor.matmul(out=acc3[:], lhsT=tps[:, 0, :],
                             rhs=w2_t[:, 0, :], start=True, stop=False)
            nc.tensor.matmul(out=acc3[:], lhsT=tps[:, 1, :],
                             rhs=w2_t[:, 1, :], start=False, stop=True)
            row = sp.tile([128, RW2], mybir.dt.bfloat16, tag="row2")
            nc.vector.tensor_copy(row[:, 0:128], acc3[:, 0:128])
            s_hi32 = sp.tile([128, 4], f32, tag="shi2")
            nc.vector.tensor_copy(row[:, 128:132], acc3[:, 128:132])
            nc.vector.tensor_copy(s_hi32[:], row[:, 128:132])
            s_lo = sp.tile([128, 4], f32, tag="slo2")
            nc.vector.tensor_tensor(out=s_lo[:], in0=acc3[:, 128:132],
                                    in1=s_hi32[:], op=ALU.subtract)
            nc.vector.tensor_copy(row[:, 132:136], s_lo[:])
            nc.sync.dma_start(t2_loc[t * 128:(t + 1) * 128, :], row[:])
            drow = sp.tile([128, 8], mybir.dt.bfloat16, tag="drow2")
            d_hi32 = sp.tile([128, 4], f32, tag="dhi2")
            nc.vector.tensor_copy(drow[:, 0:4], acc3[:, 132:136])
            nc.vector.tensor_copy(d_hi32[:], drow[:, 0:4])
            d_lo = sp.tile([128, 4], f32, tag="dlo2")
            nc.vector.tensor_tensor(out=d_lo[:], in0=acc3[:, 132:136],
                                    in1=d_hi32[:], op=ALU.subtract)
            nc.vector.tensor_copy(drow[:, 4:8], d_lo[:])
            nc.sync.dma_start(d2_loc[t * 128:(t + 1) * 128, :], drow[:])
        message_pass(tc, T1, d1_loc, RW1, 256, l1_out)

    with nc.semaphore("cc2") as cc2:
        nc.gpsimd.collective_compute(
            "AllGather", mybir.AluOpType.bypass,
            replica_groups=[list(range(NC))],
            ins=[t2_loc[:, :].opt()], outs=[T2[:, :].opt()],
        ).then_inc(cc2, 1)
        nc.gpsimd.wait_ge(cc2, 1)

    # ---------- phase 4: L2 message passing -> log_softmax -> out ----------
    with tile.TileContext(nc) as tc:
        _l2c = {}

        def l2_out(t, acc, sp, pp2):
            from concourse import mybir
            ALU = mybir.AluOpType
            AF = mybir.ActivationFunctionType
            f32 = mybir.dt.float32
            rec = sp.tile([128, 4], f32, tag="rec2")
            nc.vector.reciprocal(rec[:], acc[:, 128:132])
            o = sp.tile([128, 128], f32, tag="o")
            nc.vector.tensor_tensor(
                out=o[:].rearrange("p (h c) -> p h c", h=4),
                in0=acc[:, 0:128].rearrange("p (h c) -> p h c", h=4),
                in1=rec[:].unsqueeze(2).to_broadcast([128, 4, 32]),
                op=ALU.mult)
            if "b2" not in _l2c:
                b2_t = sp.tile([128, 128], f32, tag="b2t")
                nc.sync.dma_start(b2_t[:], b2p[0:1, :].to_broadcast([128, 128]))
                _l2c["b2"] = b2_t
            nc.vector.tensor_tensor(out=o[:], in0=o[:], in1=_l2c["b2"][:],
                                    op=ALU.add)
            # log_softmax over 128 cols
            mx = sp.tile([128, 1], f32, tag="mx")
            nc.vector.reduce_max(mx[:], o[:], axis=mybir.AxisListType.X)
            nc.vector.tensor_scalar(out=o[:], in0=o[:], scalar1=mx[:, 0:1],
                                    scalar2=None, op0=ALU.subtract)
            ex = sp.tile([128, 128], f32, tag="ex")
            nc.scalar.activation(ex[:], o[:], AF.Exp)
            sm = sp.tile([128, 1], f32, tag="sm")
            nc.vector.reduce_sum(sm[:], ex[:], axis=mybir.AxisListType.X)
            nc.scalar.activation(sm[:], sm[:], AF.Ln)
            nc.vector.tensor_scalar(out=o[:], in0=o[:], scalar1=sm[:, 0:1],
                                    scalar2=None, op0=ALU.subtract)
            # quantize to uint8: q = clamp((o + 12) * (255/12), 0, 255)
            nc.vector.tensor_scalar(out=o[:], in0=o[:], scalar1=12.0,
                                    scalar2=255.0 / 12.0, op0=ALU.add,
                                    op1=ALU.mult)
            nc.vector.tensor_scalar_max(o[:], o[:], 0.0)
            nc.vector.tensor_scalar_min(o[:], o[:], 255.0)
            o8 = sp.tile([128, 128], mybir.dt.uint8, tag="o8")
            nc.vector.tensor_copy(o8[:], o[:])
            nc.sync.dma_start(outp[t, :, :], o8[:])
        message_pass(tc, T2, d2_loc, RW2, 128, l2_out)

    return nc


def _split_sync_waits(nc, max_waits=1):
    import concourse.mybir as mybir
    ctr = [0]
    for f in nc.m.functions:
        for blk in f.blocks:
            new_list = []
            for ins in blk.instructions:
                si = ins.sync_info
                waits = list(si.on_wait) if si is not None and si.on_wait else []
                if len(waits) > max_waits:
                    keep = waits[:max_waits]
                    rest = waits[max_waits:]
                    for i in range(0, len(rest), max_waits):
                        ctr[0] += 1
                        nop = mybir.InstNoOp(
                            name=f"I-wsplit-{ctr[0]}", ins=[], outs=[],
                            engine=ins.engine)
                        nop.sync_info = mybir.SyncInfo(
                            on_wait=rest[i:i + max_waits], on_update=[])
                        new_list.append(nop)
                    ins.sync_info = mybir.SyncInfo(
                        on_wait=keep,
                        on_update=list(si.on_update) if si.on_update else [])
                new_list.append(ins)
            blk.instructions[:] = new_list


_CACHE = {}


def _get_runner():
    """Build (once) the jitted SPMD executor for the Bass program.

    Mirrors concourse.bass2jax.run_bass_via_pjrt's multi-core path, with
    three fixes: the jitted callable + on-device zero output buffers are
    cached (no per-call retrace/XLA-compile/zeros upload), outputs are
    fetched once (not once per core), and the JAX persistent compilation
    cache is enabled so fresh processes skip the walrus compile.
    """
    if "runner" in _CACHE:
        return _CACHE["runner"]
    import jax
    import jax.numpy as jnp
    from jax.sharding import Mesh, PartitionSpec
    try:
        from jax.experimental.shard_map import shard_map
    except ImportError:
        from jax import shard_map
    from concourse import mybir
    from concourse.bass2jax import (
        _bass_exec_p, install_neuronx_cc_hook, partition_id_tensor)

    try:
        jax.config.update("jax_compilation_cache_dir", "/tmp/jax_bass_cache")
        jax.config.update("jax_persistent_cache_min_compile_time_secs", 0)
        jax.config.update("jax_persistent_cache_min_entry_size_bytes", 0)
    except Exception:
        pass

    nc = _build_nc()
    _split_sync_waits(nc, 1)
    install_neuronx_cc_hook()
    assert nc.dbg_addr is None

    in_names = []
    out_names = []
    out_avals = []
    partition_name = (nc.partition_id_tensor.name
                      if nc.partition_id_tensor else None)
    for alloc in nc.m.functions[0].allocations:
        if not isinstance(alloc, mybir.MemoryLocationSet):
            continue
        name = alloc.memorylocations[0].name
        if alloc.kind == "ExternalInput":
            if name != partition_name:
                in_names.append(name)
        elif alloc.kind == "ExternalOutput":
            shape = tuple(alloc.tensor_shape)
            dtype = mybir.dt.np(alloc.dtype)
            out_names.append(name)
            out_avals.append(jax.core.ShapedArray(shape, dtype))
    n_params = len(in_names)
    full_in_names = list(in_names) + list(out_names)
    if partition_name is not None:
        full_in_names.append(partition_name)

    def _body(*args):
        operands = list(args)
        if partition_name is not None:
            operands.append(partition_id_tensor())
        outs = _bass_exec_p.bind(
            *operands,
            out_avals=tuple(out_avals),
            in_names=tuple(full_in_names),
            out_names=tuple(out_names),
            lowering_input_output_aliases=(),
            sim_require_finite=True,
            sim_require_nnan=True,
            nc=nc,
        )
        return tuple(outs)

    devices = jax.devices()[:NC]
    mesh = Mesh(np.asarray(devices), ("core",))
    n_total = n_params + len(out_names)
    sharded = jax.jit(
        shard_map(_body, mesh=mesh,
                  in_specs=(PartitionSpec("core"),) * n_total,
                  out_specs=(PartitionSpec("core"),) * len(out_names),
                  check_rep=False),
        keep_unused=True,
    )

    # zero output operands, materialized on device (never transferred)
    zfun = jax.jit(
        shard_map(
            lambda: tuple(jnp.zeros(a.shape, a.dtype) for a in out_avals),
            mesh=mesh, in_specs=(),
            out_specs=(PartitionSpec("core"),) * len(out_avals),
            check_rep=False))
    zeros = [z for z in zfun()]

    _CACHE["runner"] = (sharded, in_names, out_names, out_avals, zeros)
    return _CACHE["runner"]


def kernel(**inputs):
    import time as _time

    x = np.asarray(inputs["x"], np.float32)
    ei = np.asarray(inputs["edge_index"])
    W1a, W2a, idx_t, dl_t, dlT_t, xs, b1r, b2r = _host_prep(
        x, ei, inputs["W1"], inputs["att_src1"], inputs["att_dst1"],
        inputs["b1"], inputs["W2"], inputs["att_src2"], inputs["att_dst2"],
        inputs["b2"])

    sharded, in_names, out_names, out_avals, zeros = _get_runner()

    per_core = {
        "xT": xs, "idx": idx_t, "dl": dl_t, "dlT": dlT_t,
        "w1": np.broadcast_to(W1a, (NC,) + W1a.shape),
        "w2": np.broadcast_to(W2a, (NC,) + W2a.shape),
        "b1r": np.broadcast_to(b1r, (NC,) + b1r.shape),
        "b2r": np.broadcast_to(b2r, (NC,) + b2r.shape),
    }
    concat_in = [
        np.ascontiguousarray(per_core[name].reshape(
            NC * per_core[name].shape[1], *per_core[name].shape[2:]))
        for name in in_names
    ]

    t0 = _time.time()
    out_arrs = sharded(*concat_in, *zeros)
    res = {name: np.asarray(out_arrs[i]) for i, name in enumerate(out_names)}
    wall = _time.time() - t0
    kernel.last_wall_s = wall
    kernel.last_exec_ns = None

    q = res["out"].reshape(NC, NSHP, H * C2)
    o = q[:, :NSH].astype(np.float32) * (12.0 / 255.0) - 12.0
    kernel.last_concat_in = concat_in
    return np.ascontiguousarray(o).reshape(N, H * C2)


def measure_hw_exec_ns(iters=16):
    """Measure on-device execution time of the compiled SPMD program.

    Uploads the inputs once, then launches `iters` back-to-back executions
    (async dispatch pipelines them) and returns the amortized per-run wall
    time in ns. This approximates the neuron-profile NEFF execution time
    (upper bound: includes per-dispatch driver overhead).
    """
    import time as _time
    import jax
    from jax.sharding import Mesh, PartitionSpec, NamedSharding

    concat_in = kernel.last_concat_in
    sharded, in_names, out_names, out_avals, zeros = _get_runner()
    mesh = Mesh(np.asarray(jax.devices()[:NC]), ("core",))
    sh = NamedSharding(mesh, PartitionSpec("core"))
    dev_in = [jax.device_put(a, sh) for a in concat_in]
    jax.block_until_ready(dev_in)
    # warm (retrace for device-array args) + sanity
    out = sharded(*dev_in, *zeros)
    jax.block_until_ready(out)
    best = None
    for _ in range(5):
        t0 = _time.time()
        outs = [sharded(*dev_in, *zeros) for _ in range(iters)]
        jax.block_until_ready(outs)
        dt = (_time.time() - t0) / iters
        best = dt if best is None else min(best, dt)
    return int(best * 1e9)


# revision 43
# speedup vs baseline: 1.4337x; 1.4337x over previous
"""Trainium2 Bass kernel for 2-layer GAT (nn_GAT_50603304681766).

Strategy: partition nodes (destinations) across 8 cores. Each core:
  t1 = x_shard @ [W1 | W1@Asrc | W1@Adst]  (PE, fp8 x, fp16 weights)
  -> pack [h|s_hi|s_lo] bf16 rows -> AllGather table T1
  per dst-tile (128 nodes): gather T1[src] rows via indirect DMA,
  build one-hot scatter matrix M and its transpose MT on device
  (iota + is_equal against the per-edge dst-slot array in both
  layouts; MT uses a partition-broadcast DMA), d-expand via MT@dtab,
  g = exp(leakyrelu(s+d)), weighted scatter matmul into PSUM
  (messages + denominator), normalize, +bias, ELU, then fused
  t2 = h2 @ W2a packs table T2 -> AllGather -> layer-2 message pass
  -> log_softmax -> uint8-quantized output.
Only compact edge indices (uint16 src rows + bf16 dst slots) are
shipped to the device; the big one-hot matrices are built on-chip.
The host<->device tunnel here moves ~50 MB/s, so all transfers are
minimized: x fp8, output uint8 ([-12,0] range), weights fp16.
"""
import numpy as np
import ml_dtypes

N = 50000
E0 = 800000
F_IN = 256
H = 4
C1 = 64
C2 = 32
NEG = 0.2
NC = 8
NSH = 6250            # dst nodes per core
NSHP = 6272           # padded to 49*128
NT = 49               # dst tiles per core
NBLK = 18             # edge blocks (of 128) per dst tile
ROWS = NC * NSHP      # allgathered table rows = 50176
RW1 = 264             # T1 row: h(256) + s_hi(4) + s_lo(4)  [bf16]
RW2 = 136             # T2 row: h2'(128) + s2_hi(4) + s2_lo(4) [bf16]

bf = ml_dtypes.bfloat16


def _host_prep(x, edge_index, W1, as1, ad1, b1, W2, as2, ad2, b2):
    ei = np.asarray(edge_index)
    src = np.concatenate([ei[0], np.arange(N, dtype=ei.dtype)]).astype(np.int64)
    dst = np.concatenate([ei[1], np.arange(N, dtype=ei.dtype)]).astype(np.int64)
    ET = src.shape[0]

    # augmented weights: t = x @ [W | W@S | W@D]; s/d per head
    def aug(W, a_s, a_d, heads, ch):
        S = np.zeros((heads * ch, heads), np.float32)
        D = np.zeros((heads * ch, heads), np.float32)
        for h in range(heads):
            S[h * ch:(h + 1) * ch, h] = a_s[h]
            D[h * ch:(h + 1) * ch, h] = a_d[h]
        return np.concatenate([W, W @ S, W @ D], axis=1)  # [fin, hc+2h]

    W1a = aug(np.asarray(W1, np.float32), np.asarray(as1), np.asarray(ad1),
              H, C1).astype(np.float16)                   # [256, 264]
    W2a = aug(np.asarray(W2, np.float32), np.asarray(as2), np.asarray(ad2),
              H, C2).astype(np.float16)                   # [256, 136]

    core_of = dst // NSH
    loc = dst - core_of * NSH
    gtile = core_of * NT + loc // 128   # global dst tile id, 0..NC*NT
    dloc = loc % 128
    srow = ((src // NSH) * NSHP + (src % NSH)).astype(np.int32)

    order = np.argsort(gtile, kind="stable")
    counts = np.bincount(gtile, minlength=NC * NT)
    assert counts.max() <= NBLK * 128, f"tile overflow {counts.max()}"
    starts = np.zeros(NC * NT, np.int64)
    starts[1:] = np.cumsum(counts)[:-1]
    gs = gtile[order]
    pos = np.arange(ET, dtype=np.int64) - starts[gs]
    idx_flat = np.zeros((NC * NT, NBLK * 128), np.int32)
    idx_flat[gs, pos] = srow[order]
    dl_flat = np.full((NC * NT, NBLK * 128), 255, np.int32)
    dl_flat[gs, pos] = dloc[order]
    # per-block layout: lane p of block b <- flat[b*128 + p]
    idx_t = np.ascontiguousarray(
        idx_flat.reshape(NC, NT, NBLK, 128).transpose(0, 1, 3, 2)).astype(np.uint16)
    dl_t = np.ascontiguousarray(
        dl_flat.reshape(NC, NT, NBLK, 128).transpose(0, 1, 3, 2)).astype(bf)
    dlT_t = dl_flat.reshape(NC, NT, NBLK, 128).astype(bf)

    f8 = ml_dtypes.float8_e4m3fn
    xs = np.zeros((NC, F_IN, NSHP), f8)
    xf = np.asarray(x, np.float32).astype(f8)
    for c in range(NC):
        xs[c, :, :NSH] = xf[c * NSH:(c + 1) * NSH].T

    b1r = np.asarray(b1, np.float32)[None, :]
    b2r = np.asarray(b2, np.float32)[None, :]
    return W1a, W2a, idx_t, dl_t, dlT_t, xs, b1r, b2r


def _build_nc():
    import concourse.bass as bass
    import concourse.tile as tile
    from concourse import mybir
    from concourse.bass import IndirectOffsetOnAxis

    f32 = mybir.dt.float32
    f16 = mybir.dt.float16
    f8 = mybir.dt.float8e4
    bf16 = mybir.dt.bfloat16
    i32 = mybir.dt.int32
    u16 = mybir.dt.uint16
    AF = mybir.ActivationFunctionType
    ALU = mybir.AluOpType

    nc = bass.Bass()
    xT = nc.declare_dram_parameter("xT", [F_IN, NSHP], f8, isOutput=False)
    w1 = nc.declare_dram_parameter("w1", [F_IN, RW1], f16, isOutput=False)
    w2 = nc.declare_dram_parameter("w2", [F_IN, RW2], f16, isOutput=False)
    idxp = nc.declare_dram_parameter("idx", [NT, 128, NBLK], u16, isOutput=False)
    dlp = nc.declare_dram_parameter("dl", [NT, 128, NBLK], bf16, isOutput=False)
    dlTp = nc.declare_dram_parameter("dlT", [NT, NBLK, 128], bf16, isOutput=False)
    b1p = nc.declare_dram_parameter("b1r", [1, H * C1], f32, isOutput=False)
    b2p = nc.declare_dram_parameter("b2r", [1, H * C2], f32, isOutput=False)
    outp = nc.declare_dram_parameter("out", [NT, 128, H * C2], mybir.dt.uint8,
                                     isOutput=True)

    t1_loc = nc.dram_tensor("t1_loc", [NSHP, RW1], bf16)
    t2_loc = nc.dram_tensor("t2_loc", [NSHP, RW2], bf16)
    T1 = nc.dram_tensor("T1ag", [ROWS, RW1], bf16, addr_space="Shared")
    T2 = nc.dram_tensor("T2ag", [ROWS, RW2], bf16, addr_space="Shared")

    def message_pass(tc, Tag, dtab_all, rw, hw, sfx, out_cb):
        """hw = feature width (256 / 128); rw = table row width."""
        from concourse import mybir
        ALU = mybir.AluOpType
        AF = mybir.ActivationFunctionType
        with (
            tc.tile_pool(name="mp_c" + sfx, bufs=1) as cp,
            tc.tile_pool(name="mp_v" + sfx, bufs=3) as vp,
            tc.tile_pool(name="mp_m" + sfx, bufs=2) as mp_,
            tc.tile_pool(name="mp_s" + sfx, bufs=2) as sp,
            tc.tile_pool(name="mp_ps" + sfx, bufs=2, space="PSUM") as pp,
            tc.tile_pool(name="mp_ps2" + sfx, bufs=2, space="PSUM") as pp2,
        ):
            # constants: row-iota (f32) and per-partition iota (f32)
            eqi = cp.tile([128, 128], mybir.dt.float32)
            i2 = cp.tile([128, 128], mybir.dt.int32)
            nc.gpsimd.iota(i2[:], pattern=[[1, 128]], base=0,
                           channel_multiplier=0)
            nc.vector.tensor_copy(eqi[:], i2[:])
            iotp = cp.tile([128, 1], mybir.dt.int32)
            nc.gpsimd.iota(iotp[:], pattern=[[0, 1]], base=0,
                           channel_multiplier=1)
            iotf = cp.tile([128, 1], mybir.dt.float32)
            nc.vector.tensor_copy(iotf[:], iotp[:])
            for t in range(NT):
                idx_u = sp.tile([128, NBLK], mybir.dt.uint16, tag="idxu")
                nc.sync.dma_start(idx_u[:], idxp[t, :, :])
                idx_t = sp.tile([128, NBLK], mybir.dt.int32, tag="idx")
                nc.vector.tensor_copy(idx_t[:], idx_u[:])
                dl_t = sp.tile([128, NBLK], mybir.dt.bfloat16, tag="dl")
                nc.sync.dma_start(dl_t[:], dlp[t, :, :])
                dlf = sp.tile([128, NBLK], mybir.dt.float32, tag="dlf")
                nc.vector.tensor_copy(dlf[:], dl_t[:])
                dtab = dtab_all[:, t, :]
                v = vp.tile([128, NBLK, rw], mybir.dt.bfloat16, tag="v")
                for b in range(NBLK):
                    nc.gpsimd.indirect_dma_start(
                        out=v[:, b, :], out_offset=None, in_=Tag[:, :],
                        in_offset=IndirectOffsetOnAxis(ap=idx_t[:, b:b + 1], axis=0))
                # build one-hot M on device: M[e, (b,d)] = (dl[e,b] == d)
                m_t = mp_.tile([128, NBLK, 128], mybir.dt.bfloat16, tag="m")
                nc.vector.tensor_tensor(
                    out=m_t[:],
                    in0=dlf[:].unsqueeze(2).to_broadcast([128, NBLK, 128]),
                    in1=eqi[:].unsqueeze(1).to_broadcast([128, NBLK, 128]),
                    op=ALU.is_equal)
                # MT[d, (b,e)] = (dlT[b,e] == d): partition-broadcast DMA + cmp
                dlT_b = mp_.tile([128, NBLK, 128], mybir.dt.bfloat16, tag="dlTb")
                nc.sync.dma_start(
                    dlT_b[:],
                    dlTp[t, :, :].unsqueeze(0).to_broadcast([128, NBLK, 128]))
                mt_t = mp_.tile([128, NBLK, 128], mybir.dt.bfloat16, tag="mt")
                nc.vector.tensor_scalar(out=mt_t[:], in0=dlT_b[:],
                                        scalar1=iotf[:, 0:1], scalar2=None,
                                        op0=ALU.is_equal)
                # d-expand dex = MT @ dtab
                dex = pp2.tile([128, NBLK * 8], mybir.dt.float32, tag="dex")
                for b in range(NBLK):
                    nc.tensor.matmul(out=dex[:, b * 8:(b + 1) * 8],
                                     lhsT=mt_t[:, b, :], rhs=dtab,
                                     start=True, stop=True)
                # e = s + d (hi+lo), lrelu, exp
                s32 = sp.tile([128, NBLK, 4], mybir.dt.float32, tag="s32")
                nc.vector.tensor_tensor(out=s32[:], in0=v[:, :, hw:hw + 4],
                                        in1=v[:, :, hw + 4:hw + 8], op=ALU.add)
                dsb = sp.tile([128, NBLK, 8], mybir.dt.float32, tag="dsb")
                nc.vector.tensor_copy(dsb[:], dex[:].rearrange("p (b k) -> p b k", k=8))
                d32 = sp.tile([128, NBLK, 4], mybir.dt.float32, tag="d32")
                nc.vector.tensor_tensor(out=d32[:], in0=dsb[:, :, 0:4],
                                        in1=dsb[:, :, 4:8], op=ALU.add)
                e32 = sp.tile([128, NBLK, 4], mybir.dt.float32, tag="e32")
                nc.vector.tensor_tensor(out=e32[:], in0=s32[:], in1=d32[:],
                                        op=ALU.add)
                e_s = sp.tile([128, NBLK, 4], mybir.dt.float32, tag="es")
                nc.vector.tensor_scalar_mul(e_s[:], e32[:], NEG)
                nc.vector.tensor_tensor(out=e32[:], in0=e32[:], in1=e_s[:],
                                        op=ALU.max)
                g = sp.tile([128, NBLK, 4], mybir.dt.float32, tag="g")
                nc.scalar.activation(g[:], e32[:], AF.Exp)
                # weighted rhs [hw cols scaled by g, then g cols]
                wv = vp.tile([128, NBLK, hw + 4], mybir.dt.bfloat16, tag="wv")
                nc.vector.tensor_tensor(
                    out=wv[:, :, 0:hw].rearrange("p b (h c) -> p b h c", h=4),
                    in0=v[:, :, 0:hw].rearrange("p b (h c) -> p b h c", h=4),
                    in1=g[:].unsqueeze(3).to_broadcast([128, NBLK, 4, hw // 4]),
                    op=ALU.mult)
                nc.vector.tensor_copy(wv[:, :, hw:hw + 4], g[:])
                acc = pp.tile([128, hw + 4], mybir.dt.float32, tag="acc2")
                for b in range(NBLK):
                    nc.tensor.matmul(out=acc[:], lhsT=m_t[:, b, :],
                                     rhs=wv[:, b, :], start=(b == 0),
                                     stop=(b == NBLK - 1))
                out_cb(t, acc, sp, pp2)

    _l1c = {}
    _l2c = {}

    def l1_out(t, acc, sp, pp2):
        from concourse import mybir
        ALU = mybir.AluOpType
        AF = mybir.ActivationFunctionType
        f32 = mybir.dt.float32
        rec = sp.tile([128, 4], f32, tag="rec")
        nc.vector.reciprocal(rec[:], acc[:, 256:260])
        h2 = sp.tile([128, 256], f32, tag="h2")
        nc.vector.tensor_tensor(
            out=h2[:].rearrange("p (h c) -> p h c", h=4),
            in0=acc[:, 0:256].rearrange("p (h c) -> p h c", h=4),
            in1=rec[:].unsqueeze(2).to_broadcast([128, 4, 64]),
            op=ALU.mult)
        if "b1" not in _l1c:
            b1_t = sp.tile([128, 256], f32, tag="b1t")
            nc.sync.dma_start(b1_t[:], b1p[0:1, :].to_broadcast([128, 256]))
            _l1c["b1"] = b1_t
        nc.vector.tensor_tensor(out=h2[:], in0=h2[:], in1=_l1c["b1"][:],
                                op=ALU.add)
        # ELU: max(x, exp(min(x,0)) - 1)
        mn = sp.tile([128, 256], f32, tag="mn")
        nc.vector.tensor_scalar_min(mn[:], h2[:], 0.0)
        nc.scalar.activation(mn[:], mn[:], AF.Exp)
        nc.vector.tensor_scalar_add(mn[:], mn[:], -1.0)
        nc.vector.tensor_tensor(out=h2[:], in0=h2[:], in1=mn[:], op=ALU.max)
        # transpose h2 -> [256, 128] via PE for the fused t2 matmul
        if "idn" not in _l1c:
            idn = sp.tile([128, 128], f32, tag="idn")
            iot = sp.tile([128, 1], mybir.dt.int32, tag="iot")
            nc.gpsimd.iota(iot[:], pattern=[[0, 1]], base=0,
                           channel_multiplier=1)
            iotf = sp.tile([128, 1], f32, tag="iotf")
            nc.vector.tensor_copy(iotf[:], iot[:])
            eqi = sp.tile([128, 128], f32, tag="eqi")
            i2 = sp.tile([128, 128], mybir.dt.int32, tag="i2")
            nc.gpsimd.iota(i2[:], pattern=[[1, 128]], base=0,
                           channel_multiplier=0)
            nc.vector.tensor_copy(eqi[:], i2[:])
            nc.vector.tensor_tensor(
                out=idn[:], in0=eqi[:],
                in1=iotf[:].to_broadcast([128, 128]), op=ALU.is_equal)
            _l1c["idn"] = idn
        idn = _l1c["idn"]
        tps = sp.tile([128, 2, 128], mybir.dt.float16, tag="tps")
        for kk in range(2):
            tp = pp2.tile([128, 128], f32, tag="tp")
            nc.tensor.transpose(out=tp[:], in_=h2[:, kk * 128:(kk + 1) * 128],
                                identity=idn[:])
            nc.vector.tensor_copy(tps[:, kk, :], tp[:])
        # fused t2 = h2 @ W2a, pack T2 rows
        if "w2" not in _l1c:
            w2_t = sp.tile([128, 2, RW2], mybir.dt.float16, tag="w2t")
            nc.sync.dma_start(w2_t[:],
                              w2[:, :].rearrange("(k p) c -> p k c", p=128))
            _l1c["w2"] = w2_t
        w2_t = _l1c["w2"]
        acc3 = pp2.tile([128, RW2], f32, tag="acc3")
        nc.tensor.matmul(out=acc3[:], lhsT=tps[:, 0, :],
                         rhs=w2_t[:, 0, :], start=True, stop=False)
        nc.tensor.matmul(out=acc3[:], lhsT=tps[:, 1, :],
                         rhs=w2_t[:, 1, :], start=False, stop=True)
        row = sp.tile([128, RW2], mybir.dt.bfloat16, tag="row2")
        nc.vector.tensor_copy(row[:, 0:128], acc3[:, 0:128])
        s_hi32 = sp.tile([128, 4], f32, tag="shi2")
        nc.vector.tensor_copy(row[:, 128:132], acc3[:, 128:132])
        nc.vector.tensor_copy(s_hi32[:], row[:, 128:132])
        s_lo = sp.tile([128, 4], f32, tag="slo2")
        nc.vector.tensor_tensor(out=s_lo[:], in0=acc3[:, 128:132],
                                in1=s_hi32[:], op=ALU.subtract)
        nc.vector.tensor_copy(row[:, 132:136], s_lo[:])
        nc.sync.dma_start(
            t2_loc[t * 128:(t + 1) * 128, :], row[:])
        # d2 hi/lo straight into the SBUF-resident table
        d_hi32 = sp.tile([128, 4], f32, tag="dhi2")
        nc.vector.tensor_copy(dtab2[:, t, 0:4], acc3[:, 132:136])
        nc.vector.tensor_copy(d_hi32[:], dtab2[:, t, 0:4])
        d_lo = sp.tile([128, 4], f32, tag="dlo2")
        nc.vector.tensor_tensor(out=d_lo[:], in0=acc3[:, 132:136],
                                in1=d_hi32[:], op=ALU.subtract)
        nc.vector.tensor_copy(dtab2[:, t, 4:8], d_lo[:])

    def l2_out(t, acc, sp, pp2):
        from concourse import mybir
        ALU = mybir.AluOpType
        AF = mybir.ActivationFunctionType
        f32 = mybir.dt.float32
        rec = sp.tile([128, 4], f32, tag="rec2")
        nc.vector.reciprocal(rec[:], acc[:, 128:132])
        o = sp.tile([128, 128], f32, tag="o")
        nc.vector.tensor_tensor(
            out=o[:].rearrange("p (h c) -> p h c", h=4),
            in0=acc[:, 0:128].rearrange("p (h c) -> p h c", h=4),
            in1=rec[:].unsqueeze(2).to_broadcast([128, 4, 32]),
            op=ALU.mult)
        if "b2" not in _l2c:
            b2_t = sp.tile([128, 128], f32, tag="b2t")
            nc.sync.dma_start(b2_t[:], b2p[0:1, :].to_broadcast([128, 128]))
            _l2c["b2"] = b2_t
        nc.vector.tensor_tensor(out=o[:], in0=o[:], in1=_l2c["b2"][:],
                                op=ALU.add)
        # log_softmax over 128 cols
        mx = sp.tile([128, 1], f32, tag="mx")
        nc.vector.reduce_max(mx[:], o[:], axis=mybir.AxisListType.X)
        nc.vector.tensor_scalar(out=o[:], in0=o[:], scalar1=mx[:, 0:1],
                                scalar2=None, op0=ALU.subtract)
        ex = sp.tile([128, 128], f32, tag="ex")
        nc.scalar.activation(ex[:], o[:], AF.Exp)
        sm = sp.tile([128, 1], f32, tag="sm")
        nc.vector.reduce_sum(sm[:], ex[:], axis=mybir.AxisListType.X)
        nc.scalar.activation(sm[:], sm[:], AF.Ln)
        nc.vector.tensor_scalar(out=o[:], in0=o[:], scalar1=sm[:, 0:1],
                                scalar2=None, op0=ALU.subtract)
        # quantize to uint8: q = clamp((o + 12) * (255/12), 0, 255)
        nc.vector.tensor_scalar(out=o[:], in0=o[:], scalar1=12.0,
                                scalar2=255.0 / 12.0, op0=ALU.add,
                                op1=ALU.mult)
        nc.vector.tensor_scalar_max(o[:], o[:], 0.0)
        nc.vector.tensor_scalar_min(o[:], o[:], 255.0)
        o8 = sp.tile([128, 128], mybir.dt.uint8, tag="o8")
        nc.vector.tensor_copy(o8[:], o[:])
        nc.sync.dma_start(outp[t, :, :], o8[:])

    # single TileContext: phase barriers replaced by semaphore-gated
    # AllGathers; d-tables live in SBUF for the whole program
    with (
        tile.TileContext(nc) as tc,
        tc.tile_pool(name="glob", bufs=1) as gp,
    ):
        dtab1 = gp.tile([128, NT, 8], bf16)
        dtab2 = gp.tile([128, NT, 8], bf16)
        # ---------- phase 1: t1 = xT.T @ W1a ; pack tables ----------
        with (
            tc.tile_pool(name="w", bufs=1) as wp,
            tc.tile_pool(name="a", bufs=3) as ap,
            tc.tile_pool(name="ps", bufs=2, space="PSUM") as pp,
        ):
            w1_t = wp.tile([128, 2, RW1], f16)
            nc.sync.dma_start(w1_t[:], w1[:, :].rearrange("(k p) c -> p k c", p=128))
            for t in range(NT):
                xt8 = ap.tile([128, 2, 128], f8, tag="xt8")
                nc.sync.dma_start(
                    xt8[:],
                    xT[:, t * 128:(t + 1) * 128].rearrange("(k p) c -> p k c", p=128))
                xt = ap.tile([128, 2, 128], f16, tag="xt")
                nc.vector.tensor_copy(xt[:], xt8[:])
                acc = pp.tile([128, RW1], f32, tag="acc")
                nc.tensor.matmul(out=acc[:], lhsT=xt[:, 0, :],
                                 rhs=w1_t[:, 0, :], start=True, stop=False)
                nc.tensor.matmul(out=acc[:], lhsT=xt[:, 1, :],
                                 rhs=w1_t[:, 1, :], start=False, stop=True)
                # pack row: h bf16, s hi/lo
                row = ap.tile([128, RW1], bf16, tag="row")
                nc.vector.tensor_copy(row[:, 0:256], acc[:, 0:256])
                s_hi32 = ap.tile([128, 4], f32, tag="shi32")
                nc.vector.tensor_copy(row[:, 256:260], acc[:, 256:260])
                nc.vector.tensor_copy(s_hi32[:], row[:, 256:260])
                s_lo = ap.tile([128, 4], f32, tag="slo")
                nc.vector.tensor_tensor(out=s_lo[:], in0=acc[:, 256:260],
                                        in1=s_hi32[:], op=ALU.subtract)
                nc.vector.tensor_copy(row[:, 260:264], s_lo[:])
                nc.sync.dma_start(
                    t1_loc[t * 128:(t + 1) * 128, :], row[:])
                # d1 hi/lo straight into the SBUF-resident table
                d_hi32 = ap.tile([128, 4], f32, tag="dhi32")
                nc.vector.tensor_copy(dtab1[:, t, 0:4], acc[:, 260:264])
                nc.vector.tensor_copy(d_hi32[:], dtab1[:, t, 0:4])
                d_lo = ap.tile([128, 4], f32, tag="dlo")
                nc.vector.tensor_tensor(out=d_lo[:], in0=acc[:, 260:264],
                                        in1=d_hi32[:], op=ALU.subtract)
                nc.vector.tensor_copy(dtab1[:, t, 4:8], d_lo[:])
        nc.gpsimd.collective_compute(
            "AllGather", mybir.AluOpType.bypass,
            replica_groups=[list(range(NC))],
            ins=[t1_loc[:, :].opt()], outs=[T1[:, :].opt()],
        )
        # ---------- L1 message passing -> h2 -> fused t2, pack T2 ----------
        message_pass(tc, T1, dtab1, RW1, 256, "a", l1_out)
        nc.gpsimd.collective_compute(
            "AllGather", mybir.AluOpType.bypass,
            replica_groups=[list(range(NC))],
            ins=[t2_loc[:, :].opt()], outs=[T2[:, :].opt()],
        )
        # ---------- L2 message passing -> log_softmax -> out ----------
        message_pass(tc, T2, dtab2, RW2, 128, "b", l2_out)

    return nc


def _split_sync_waits(nc, max_waits=1):
    import concourse.mybir as mybir
    ctr = [0]
    for f in nc.m.functions:
        for blk in f.blocks:
            new_list = []
            for ins in blk.instructions:
                si = ins.sync_info
                waits = list(si.on_wait) if si is not None and si.on_wait else []
                if len(waits) > max_waits:
                    keep = waits[:max_waits]
                    rest = waits[max_waits:]
                    for i in range(0, len(rest), max_waits):
                        ctr[0] += 1
                        nop = mybir.InstNoOp(
                            name=f"I-wsplit-{ctr[0]}", ins=[], outs=[],
                            engine=ins.engine)
                        nop.sync_info = mybir.SyncInfo(
                            on_wait=rest[i:i + max_waits], on_update=[])
                        new_list.append(nop)
                    ins.sync_info = mybir.SyncInfo(
                        on_wait=keep,
                        on_update=list(si.on_update) if si.on_update else [])
                new_list.append(ins)
            blk.instructions[:] = new_list


_CACHE = {}


def _get_runner():
    """Build (once) the jitted SPMD executor for the Bass program.

    Mirrors concourse.bass2jax.run_bass_via_pjrt's multi-core path, with
    three fixes: the jitted callable + on-device zero output buffers are
    cached (no per-call retrace/XLA-compile/zeros upload), outputs are
    fetched once (not once per core), and the JAX persistent compilation
    cache is enabled so fresh processes skip the walrus compile.
    """
    if "runner" in _CACHE:
        return _CACHE["runner"]
    import jax
    import jax.numpy as jnp
    from jax.sharding import Mesh, PartitionSpec
    try:
        from jax.experimental.shard_map import shard_map
    except ImportError:
        from jax import shard_map
    from concourse import mybir
    from concourse.bass2jax import (
        _bass_exec_p, install_neuronx_cc_hook, partition_id_tensor)

    try:
        jax.config.update("jax_compilation_cache_dir", "/tmp/jax_bass_cache")
        jax.config.update("jax_persistent_cache_min_compile_time_secs", 0)
        jax.config.update("jax_persistent_cache_min_entry_size_bytes", 0)
    except Exception:
        pass

    nc = _build_nc()
    _split_sync_waits(nc, 1)
    install_neuronx_cc_hook()
    assert nc.dbg_addr is None

    in_names = []
    out_names = []
    out_avals = []
    partition_name = (nc.partition_id_tensor.name
                      if nc.partition_id_tensor else None)
    for alloc in nc.m.functions[0].allocations:
        if not isinstance(alloc, mybir.MemoryLocationSet):
            continue
        name = alloc.memorylocations[0].name
        if alloc.kind == "ExternalInput":
            if name != partition_name:
                in_names.append(name)
        elif alloc.kind == "ExternalOutput":
            shape = tuple(alloc.tensor_shape)
            dtype = mybir.dt.np(alloc.dtype)
            out_names.append(name)
            out_avals.append(jax.core.ShapedArray(shape, dtype))
    n_params = len(in_names)
    full_in_names = list(in_names) + list(out_names)
    if partition_name is not None:
        full_in_names.append(partition_name)

    def _body(*args):
        operands = list(args)
        if partition_name is not None:
            operands.append(partition_id_tensor())
        outs = _bass_exec_p.bind(
            *operands,
            out_avals=tuple(out_avals),
            in_names=tuple(full_in_names),
            out_names=tuple(out_names),
            lowering_input_output_aliases=(),
            sim_require_finite=True,
            sim_require_nnan=True,
            nc=nc,
        )
        return tuple(outs)

    devices = jax.devices()[:NC]
    mesh = Mesh(np.asarray(devices), ("core",))
    n_total = n_params + len(out_names)
    sharded = jax.jit(
        shard_map(_body, mesh=mesh,
                  in_specs=(PartitionSpec("core"),) * n_total,
                  out_specs=(PartitionSpec("core"),) * len(out_names),
                  check_rep=False),
        keep_unused=True,
    )

    # zero output operands, materialized on device (never transferred)
    zfun = jax.jit(
        shard_map(
            lambda: tuple(jnp.zeros(a.shape, a.dtype) for a in out_avals),
            mesh=mesh, in_specs=(),
            out_specs=(PartitionSpec("core"),) * len(out_avals),
            check_rep=False))
    zeros = [z for z in zfun()]

    _CACHE["runner"] = (sharded, in_names, out_names, out_avals, zeros)
    return _CACHE["runner"]


def kernel(**inputs):
    import time as _time

    x = np.asarray(inputs["x"], np.float32)
    ei = np.asarray(inputs["edge_index"])
    W1a, W2a, idx_t, dl_t, dlT_t, xs, b1r, b2r = _host_prep(
        x, ei, inputs["W1"], inputs["att_src1"], inputs["att_dst1"],
        inputs["b1"], inputs["W2"], inputs["att_src2"], inputs["att_dst2"],
        inputs["b2"])

    sharded, in_names, out_names, out_avals, zeros = _get_runner()

    per_core = {
        "xT": xs, "idx": idx_t, "dl": dl_t, "dlT": dlT_t,
        "w1": np.broadcast_to(W1a, (NC,) + W1a.shape),
        "w2": np.broadcast_to(W2a, (NC,) + W2a.shape),
        "b1r": np.broadcast_to(b1r, (NC,) + b1r.shape),
        "b2r": np.broadcast_to(b2r, (NC,) + b2r.shape),
    }
    concat_in = [
        np.ascontiguousarray(per_core[name].reshape(
            NC * per_core[name].shape[1], *per_core[name].shape[2:]))
        for name in in_names
    ]

    t0 = _time.time()
    out_arrs = sharded(*concat_in, *zeros)
    res = {name: np.asarray(out_arrs[i]) for i, name in enumerate(out_names)}
    wall = _time.time() - t0
    kernel.last_wall_s = wall
    kernel.last_exec_ns = None

    q = res["out"].reshape(NC, NSHP, H * C2)
    o = q[:, :NSH].astype(np.float32) * (12.0 / 255.0) - 12.0
    kernel.last_concat_in = concat_in
    return np.ascontiguousarray(o).reshape(N, H * C2)


def measure_hw_exec_ns(iters=16):
    """Measure on-device execution time of the compiled SPMD program.

    Uploads the inputs once, then launches `iters` back-to-back executions
    (async dispatch pipelines them) and returns the amortized per-run wall
    time in ns. This approximates the neuron-profile NEFF execution time
    (upper bound: includes per-dispatch driver overhead).
    """
    import time as _time
    import jax
    from jax.sharding import Mesh, PartitionSpec, NamedSharding

    concat_in = kernel.last_concat_in
    sharded, in_names, out_names, out_avals, zeros = _get_runner()
    mesh = Mesh(np.asarray(jax.devices()[:NC]), ("core",))
    sh = NamedSharding(mesh, PartitionSpec("core"))
    dev_in = [jax.device_put(a, sh) for a in concat_in]
    jax.block_until_ready(dev_in)
    # warm (retrace for device-array args) + sanity
    out = sharded(*dev_in, *zeros)
    jax.block_until_ready(out)
    best = None
    for _ in range(5):
        t0 = _time.time()
        outs = [sharded(*dev_in, *zeros) for _ in range(iters)]
        jax.block_until_ready(outs)
        dt = (_time.time() - t0) / iters
        best = dt if best is None else min(best, dt)
    return int(best * 1e9)


# revision 45
# speedup vs baseline: 1.5552x; 1.0847x over previous
"""Trainium2 Bass kernel for 2-layer GAT (nn_GAT_50603304681766).

Strategy: partition nodes (destinations) across 8 cores. Each core:
  t1 = x_shard @ [W1 | W1@Asrc | W1@Adst]  (PE, fp8 x, fp16 weights)
  -> pack [h|s_hi|s_lo] bf16 rows -> AllGather table T1
  per dst-tile (128 nodes): gather T1[src] rows via indirect DMA,
  build one-hot scatter matrix M and its transpose MT on device
  (iota + is_equal against the per-edge dst-slot array in both
  layouts; MT uses a partition-broadcast DMA), d-expand via MT@dtab,
  g = exp(leakyrelu(s+d)), weighted scatter matmul into PSUM
  (messages + denominator), normalize, +bias, ELU, then fused
  t2 = h2 @ W2a packs table T2 -> AllGather -> layer-2 message pass
  -> log_softmax -> uint8-quantized output.
Only compact edge indices (uint16 src rows + bf16 dst slots) are
shipped to the device; the big one-hot matrices are built on-chip.
The host<->device tunnel here moves ~50 MB/s, so all transfers are
minimized: x fp8, output uint8 ([-12,0] range), weights fp16.
"""
import numpy as np
import ml_dtypes

N = 50000
E0 = 800000
F_IN = 256
H = 4
C1 = 64
C2 = 32
NEG = 0.2
NC = 8
NSH = 6250            # dst nodes per core
NSHP = 6272           # padded to 49*128
NT = 49               # dst tiles per core
NBLK = 18             # edge blocks (of 128) per dst tile
ROWS = NC * NSHP      # allgathered table rows = 50176
RW1 = 264             # T1 row: h(256) + s_hi(4) + s_lo(4)  [bf16]
RW2 = 136             # T2 row: h2'(128) + s2_hi(4) + s2_lo(4) [bf16]

bf = ml_dtypes.bfloat16


def _host_prep(x, edge_index, W1, as1, ad1, b1, W2, as2, ad2, b2):
    ei = np.asarray(edge_index)
    src = np.concatenate([ei[0], np.arange(N, dtype=ei.dtype)]).astype(np.int64)
    dst = np.concatenate([ei[1], np.arange(N, dtype=ei.dtype)]).astype(np.int64)
    ET = src.shape[0]

    # augmented weights: t = x @ [W | W@S | W@D]; s/d per head
    def aug(W, a_s, a_d, heads, ch):
        S = np.zeros((heads * ch, heads), np.float32)
        D = np.zeros((heads * ch, heads), np.float32)
        for h in range(heads):
            S[h * ch:(h + 1) * ch, h] = a_s[h]
            D[h * ch:(h + 1) * ch, h] = a_d[h]
        return np.concatenate([W, W @ S, W @ D], axis=1)  # [fin, hc+2h]

    W1a = aug(np.asarray(W1, np.float32), np.asarray(as1), np.asarray(ad1),
              H, C1).astype(np.float16)                   # [256, 264]
    W2a = aug(np.asarray(W2, np.float32), np.asarray(as2), np.asarray(ad2),
              H, C2).astype(np.float16)                   # [256, 136]

    core_of = dst // NSH
    loc = dst - core_of * NSH
    gtile = core_of * NT + loc // 128   # global dst tile id, 0..NC*NT
    dloc = loc % 128
    srow = ((src // NSH) * NSHP + (src % NSH)).astype(np.int32)

    order = np.argsort(gtile, kind="stable")
    counts = np.bincount(gtile, minlength=NC * NT)
    assert counts.max() <= NBLK * 128, f"tile overflow {counts.max()}"
    starts = np.zeros(NC * NT, np.int64)
    starts[1:] = np.cumsum(counts)[:-1]
    gs = gtile[order]
    pos = np.arange(ET, dtype=np.int64) - starts[gs]
    idx_flat = np.zeros((NC * NT, NBLK * 128), np.int32)
    idx_flat[gs, pos] = srow[order]
    dl_flat = np.full((NC * NT, NBLK * 128), 255, np.int32)
    dl_flat[gs, pos] = dloc[order]
    # per-block layout: lane p of block b <- flat[b*128 + p]
    idx_t = np.ascontiguousarray(
        idx_flat.reshape(NC, NT, NBLK, 128).transpose(0, 1, 3, 2)).astype(np.uint16)
    dl_t = np.ascontiguousarray(
        dl_flat.reshape(NC, NT, NBLK, 128).transpose(0, 1, 3, 2)).astype(bf)
    dlT_t = dl_flat.reshape(NC, NT, NBLK, 128).astype(bf)

    f8 = ml_dtypes.float8_e4m3fn
    xs = np.zeros((NC, F_IN, NSHP), f8)
    xf = np.asarray(x, np.float32).astype(f8)
    for c in range(NC):
        xs[c, :, :NSH] = xf[c * NSH:(c + 1) * NSH].T

    b1r = np.asarray(b1, np.float32)[None, :]
    b2r = np.asarray(b2, np.float32)[None, :]
    return W1a, W2a, idx_t, dl_t, dlT_t, xs, b1r, b2r


def _build_nc():
    import concourse.bass as bass
    import concourse.tile as tile
    from concourse import mybir
    from concourse.bass import IndirectOffsetOnAxis

    f32 = mybir.dt.float32
    f16 = mybir.dt.float16
    f8 = mybir.dt.float8e4
    bf16 = mybir.dt.bfloat16
    i32 = mybir.dt.int32
    u16 = mybir.dt.uint16
    AF = mybir.ActivationFunctionType
    ALU = mybir.AluOpType

    nc = bass.Bass()
    xT = nc.declare_dram_parameter("xT", [F_IN, NSHP], f8, isOutput=False)
    w1 = nc.declare_dram_parameter("w1", [F_IN, RW1], f16, isOutput=False)
    w2 = nc.declare_dram_parameter("w2", [F_IN, RW2], f16, isOutput=False)
    idxp = nc.declare_dram_parameter("idx", [NT, 128, NBLK], u16, isOutput=False)
    dlp = nc.declare_dram_parameter("dl", [NT, 128, NBLK], bf16, isOutput=False)
    dlTp = nc.declare_dram_parameter("dlT", [NT, NBLK, 128], bf16, isOutput=False)
    b1p = nc.declare_dram_parameter("b1r", [1, H * C1], f32, isOutput=False)
    b2p = nc.declare_dram_parameter("b2r", [1, H * C2], f32, isOutput=False)
    outp = nc.declare_dram_parameter("out", [NT, 128, H * C2], mybir.dt.uint8,
                                     isOutput=True)

    t1_loc = nc.dram_tensor("t1_loc", [NSHP, RW1], bf16)
    t2_loc = nc.dram_tensor("t2_loc", [NSHP, RW2], bf16)
    T1 = nc.dram_tensor("T1ag", [ROWS, RW1], bf16, addr_space="Shared")
    T2 = nc.dram_tensor("T2ag", [ROWS, RW2], bf16, addr_space="Shared")

    def message_pass(tc, Tag, dtab_all, rw, hw, sfx, out_cb):
        """hw = feature width (256 / 128); rw = table row width."""
        from concourse import mybir
        ALU = mybir.AluOpType
        AF = mybir.ActivationFunctionType
        with (
            tc.tile_pool(name="mp_c" + sfx, bufs=1) as cp,
            tc.tile_pool(name="mp_v" + sfx, bufs=3) as vp,
            tc.tile_pool(name="mp_m" + sfx, bufs=2) as mp_,
            tc.tile_pool(name="mp_s" + sfx, bufs=2) as sp,
            tc.tile_pool(name="mp_ps" + sfx, bufs=2, space="PSUM") as pp,
            tc.tile_pool(name="mp_ps2" + sfx, bufs=2, space="PSUM") as pp2,
        ):
            # constants: row-iota (f32) and per-partition iota (f32)
            eqi = cp.tile([128, 128], mybir.dt.float32)
            i2 = cp.tile([128, 128], mybir.dt.int32)
            nc.gpsimd.iota(i2[:], pattern=[[1, 128]], base=0,
                           channel_multiplier=0)
            nc.vector.tensor_copy(eqi[:], i2[:])
            iotp = cp.tile([128, 1], mybir.dt.int32)
            nc.gpsimd.iota(iotp[:], pattern=[[0, 1]], base=0,
                           channel_multiplier=1)
            iotf = cp.tile([128, 1], mybir.dt.float32)
            nc.vector.tensor_copy(iotf[:], iotp[:])
            # hoisted: all idx/dl tiles loaded + converted once per pass
            idx_u = cp.tile([128, NT, NBLK], mybir.dt.uint16)
            nc.sync.dma_start(idx_u[:], idxp[:, :, :].rearrange("t p b -> p t b"))
            idx_all = cp.tile([128, NT, NBLK], mybir.dt.int32)
            nc.vector.tensor_copy(idx_all[:], idx_u[:])
            dl_u = cp.tile([128, NT, NBLK], mybir.dt.bfloat16)
            nc.sync.dma_start(dl_u[:], dlp[:, :, :].rearrange("t p b -> p t b"))
            dlf_all = cp.tile([128, NT, NBLK], mybir.dt.float32)
            nc.vector.tensor_copy(dlf_all[:], dl_u[:])
            for t in range(NT):
                idx_t = idx_all[:, t, :]
                dlf = dlf_all[:, t, :]
                dtab = dtab_all[:, t, :]
                v = vp.tile([128, NBLK, rw], mybir.dt.bfloat16, tag="v")
                for b in range(NBLK):
                    nc.gpsimd.indirect_dma_start(
                        out=v[:, b, :], out_offset=None, in_=Tag[:, :],
                        in_offset=IndirectOffsetOnAxis(ap=idx_t[:, b:b + 1], axis=0))
                # build one-hot M on device: M[e, (b,d)] = (dl[e,b] == d)
                m_t = mp_.tile([128, NBLK, 128], mybir.dt.bfloat16, tag="m")
                nc.vector.tensor_tensor(
                    out=m_t[:],
                    in0=dlf.unsqueeze(2).to_broadcast([128, NBLK, 128]),
                    in1=eqi[:].unsqueeze(1).to_broadcast([128, NBLK, 128]),
                    op=ALU.is_equal)
                # MT[d, (b,e)] = (dlT[b,e] == d): partition-broadcast DMA + cmp
                dlT_b = mp_.tile([128, NBLK, 128], mybir.dt.bfloat16, tag="dlTb")
                nc.sync.dma_start(
                    dlT_b[:],
                    dlTp[t, :, :].unsqueeze(0).to_broadcast([128, NBLK, 128]))
                mt_t = mp_.tile([128, NBLK, 128], mybir.dt.bfloat16, tag="mt")
                nc.vector.tensor_scalar(out=mt_t[:], in0=dlT_b[:],
                                        scalar1=iotf[:, 0:1], scalar2=None,
                                        op0=ALU.is_equal)
                # d-expand dex = MT @ dtab
                dex = pp2.tile([128, NBLK * 8], mybir.dt.float32, tag="dex")
                for b in range(NBLK):
                    nc.tensor.matmul(out=dex[:, b * 8:(b + 1) * 8],
                                     lhsT=mt_t[:, b, :], rhs=dtab,
                                     start=True, stop=True)
                # e = s + d (hi+lo), lrelu, exp
                s32 = sp.tile([128, NBLK, 4], mybir.dt.float32, tag="s32")
                nc.vector.tensor_tensor(out=s32[:], in0=v[:, :, hw:hw + 4],
                                        in1=v[:, :, hw + 4:hw + 8], op=ALU.add)
                dsb = sp.tile([128, NBLK, 8], mybir.dt.float32, tag="dsb")
                nc.vector.tensor_copy(dsb[:], dex[:].rearrange("p (b k) -> p b k", k=8))
                d32 = sp.tile([128, NBLK, 4], mybir.dt.float32, tag="d32")
                nc.vector.tensor_tensor(out=d32[:], in0=dsb[:, :, 0:4],
                                        in1=dsb[:, :, 4:8], op=ALU.add)
                e32 = sp.tile([128, NBLK, 4], mybir.dt.float32, tag="e32")
                nc.vector.tensor_tensor(out=e32[:], in0=s32[:], in1=d32[:],
                                        op=ALU.add)
                e_s = sp.tile([128, NBLK, 4], mybir.dt.float32, tag="es")
                nc.vector.tensor_scalar_mul(e_s[:], e32[:], NEG)
                nc.vector.tensor_tensor(out=e32[:], in0=e32[:], in1=e_s[:],
                                        op=ALU.max)
                g = sp.tile([128, NBLK, 4], mybir.dt.float32, tag="g")
                nc.scalar.activation(g[:], e32[:], AF.Exp)
                # weighted rhs [hw cols scaled by g, then g cols]
                wv = vp.tile([128, NBLK, hw + 4], mybir.dt.bfloat16, tag="wv")
                nc.vector.tensor_tensor(
                    out=wv[:, :, 0:hw].rearrange("p b (h c) -> p b h c", h=4),
                    in0=v[:, :, 0:hw].rearrange("p b (h c) -> p b h c", h=4),
                    in1=g[:].unsqueeze(3).to_broadcast([128, NBLK, 4, hw // 4]),
                    op=ALU.mult)
                nc.vector.tensor_copy(wv[:, :, hw:hw + 4], g[:])
                acc = pp.tile([128, hw + 4], mybir.dt.float32, tag="acc2")
                for b in range(NBLK):
                    nc.tensor.matmul(out=acc[:], lhsT=m_t[:, b, :],
                                     rhs=wv[:, b, :], start=(b == 0),
                                     stop=(b == NBLK - 1))
                out_cb(t, acc, sp, pp2)

    _l1c = {}
    _l2c = {}

    def l1_out(t, acc, sp, pp2):
        from concourse import mybir
        ALU = mybir.AluOpType
        AF = mybir.ActivationFunctionType
        f32 = mybir.dt.float32
        rec = sp.tile([128, 4], f32, tag="rec")
        nc.vector.reciprocal(rec[:], acc[:, 256:260])
        h2 = sp.tile([128, 256], f32, tag="h2")
        nc.vector.tensor_tensor(
            out=h2[:].rearrange("p (h c) -> p h c", h=4),
            in0=acc[:, 0:256].rearrange("p (h c) -> p h c", h=4),
            in1=rec[:].unsqueeze(2).to_broadcast([128, 4, 64]),
            op=ALU.mult)
        if "b1" not in _l1c:
            b1_t = sp.tile([128, 256], f32, tag="b1t")
            nc.sync.dma_start(b1_t[:], b1p[0:1, :].to_broadcast([128, 256]))
            _l1c["b1"] = b1_t
        nc.vector.tensor_tensor(out=h2[:], in0=h2[:], in1=_l1c["b1"][:],
                                op=ALU.add)
        # ELU: max(x, exp(min(x,0)) - 1)
        mn = sp.tile([128, 256], f32, tag="mn")
        nc.vector.tensor_scalar_min(mn[:], h2[:], 0.0)
        nc.scalar.activation(mn[:], mn[:], AF.Exp)
        nc.vector.tensor_scalar_add(mn[:], mn[:], -1.0)
        nc.vector.tensor_tensor(out=h2[:], in0=h2[:], in1=mn[:], op=ALU.max)
        # transpose h2 -> [256, 128] via PE for the fused t2 matmul
        if "idn" not in _l1c:
            idn = sp.tile([128, 128], f32, tag="idn")
            iot = sp.tile([128, 1], mybir.dt.int32, tag="iot")
            nc.gpsimd.iota(iot[:], pattern=[[0, 1]], base=0,
                           channel_multiplier=1)
            iotf = sp.tile([128, 1], f32, tag="iotf")
            nc.vector.tensor_copy(iotf[:], iot[:])
            eqi = sp.tile([128, 128], f32, tag="eqi")
            i2 = sp.tile([128, 128], mybir.dt.int32, tag="i2")
            nc.gpsimd.iota(i2[:], pattern=[[1, 128]], base=0,
                           channel_multiplier=0)
            nc.vector.tensor_copy(eqi[:], i2[:])
            nc.vector.tensor_tensor(
                out=idn[:], in0=eqi[:],
                in1=iotf[:].to_broadcast([128, 128]), op=ALU.is_equal)
            _l1c["idn"] = idn
        idn = _l1c["idn"]
        tps = sp.tile([128, 2, 128], mybir.dt.float16, tag="tps")
        for kk in range(2):
            tp = pp2.tile([128, 128], f32, tag="tp")
            nc.tensor.transpose(out=tp[:], in_=h2[:, kk * 128:(kk + 1) * 128],
                                identity=idn[:])
            nc.vector.tensor_copy(tps[:, kk, :], tp[:])
        # fused t2 = h2 @ W2a, pack T2 rows
        if "w2" not in _l1c:
            w2_t = sp.tile([128, 2, RW2], mybir.dt.float16, tag="w2t")
            nc.sync.dma_start(w2_t[:],
                              w2[:, :].rearrange("(k p) c -> p k c", p=128))
            _l1c["w2"] = w2_t
        w2_t = _l1c["w2"]
        acc3 = pp2.tile([128, RW2], f32, tag="acc3")
        nc.tensor.matmul(out=acc3[:], lhsT=tps[:, 0, :],
                         rhs=w2_t[:, 0, :], start=True, stop=False)
        nc.tensor.matmul(out=acc3[:], lhsT=tps[:, 1, :],
                         rhs=w2_t[:, 1, :], start=False, stop=True)
        row = sp.tile([128, RW2], mybir.dt.bfloat16, tag="row2")
        nc.vector.tensor_copy(row[:, 0:128], acc3[:, 0:128])
        s_hi32 = sp.tile([128, 4], f32, tag="shi2")
        nc.vector.tensor_copy(row[:, 128:132], acc3[:, 128:132])
        nc.vector.tensor_copy(s_hi32[:], row[:, 128:132])
        s_lo = sp.tile([128, 4], f32, tag="slo2")
        nc.vector.tensor_tensor(out=s_lo[:], in0=acc3[:, 128:132],
                                in1=s_hi32[:], op=ALU.subtract)
        nc.vector.tensor_copy(row[:, 132:136], s_lo[:])
        nc.sync.dma_start(
            t2_loc[t * 128:(t + 1) * 128, :], row[:])
        # d2 hi/lo straight into the SBUF-resident table
        d_hi32 = sp.tile([128, 4], f32, tag="dhi2")
        nc.vector.tensor_copy(dtab2[:, t, 0:4], acc3[:, 132:136])
        nc.vector.tensor_copy(d_hi32[:], dtab2[:, t, 0:4])
        d_lo = sp.tile([128, 4], f32, tag="dlo2")
        nc.vector.tensor_tensor(out=d_lo[:], in0=acc3[:, 132:136],
                                in1=d_hi32[:], op=ALU.subtract)
        nc.vector.tensor_copy(dtab2[:, t, 4:8], d_lo[:])

    def l2_out(t, acc, sp, pp2):
        from concourse import mybir
        ALU = mybir.AluOpType
        AF = mybir.ActivationFunctionType
        f32 = mybir.dt.float32
        rec = sp.tile([128, 4], f32, tag="rec2")
        nc.vector.reciprocal(rec[:], acc[:, 128:132])
        o = sp.tile([128, 128], f32, tag="o")
        nc.vector.tensor_tensor(
            out=o[:].rearrange("p (h c) -> p h c", h=4),
            in0=acc[:, 0:128].rearrange("p (h c) -> p h c", h=4),
            in1=rec[:].unsqueeze(2).to_broadcast([128, 4, 32]),
            op=ALU.mult)
        if "b2" not in _l2c:
            b2_t = sp.tile([128, 128], f32, tag="b2t")
            nc.sync.dma_start(b2_t[:], b2p[0:1, :].to_broadcast([128, 128]))
            _l2c["b2"] = b2_t
        nc.vector.tensor_tensor(out=o[:], in0=o[:], in1=_l2c["b2"][:],
                                op=ALU.add)
        # log_softmax over 128 cols
        mx = sp.tile([128, 1], f32, tag="mx")
        nc.vector.reduce_max(mx[:], o[:], axis=mybir.AxisListType.X)
        nc.vector.tensor_scalar(out=o[:], in0=o[:], scalar1=mx[:, 0:1],
                                scalar2=None, op0=ALU.subtract)
        ex = sp.tile([128, 128], f32, tag="ex")
        nc.scalar.activation(ex[:], o[:], AF.Exp)
        sm = sp.tile([128, 1], f32, tag="sm")
        nc.vector.reduce_sum(sm[:], ex[:], axis=mybir.AxisListType.X)
        nc.scalar.activation(sm[:], sm[:], AF.Ln)
        nc.vector.tensor_scalar(out=o[:], in0=o[:], scalar1=sm[:, 0:1],
                                scalar2=None, op0=ALU.subtract)
        # quantize to uint8: q = clamp((o + 12) * (255/12), 0, 255)
        nc.vector.tensor_scalar(out=o[:], in0=o[:], scalar1=12.0,
                                scalar2=255.0 / 12.0, op0=ALU.add,
                                op1=ALU.mult)
        nc.vector.tensor_scalar_max(o[:], o[:], 0.0)
        nc.vector.tensor_scalar_min(o[:], o[:], 255.0)
        o8 = sp.tile([128, 128], mybir.dt.uint8, tag="o8")
        nc.vector.tensor_copy(o8[:], o[:])
        nc.sync.dma_start(outp[t, :, :], o8[:])

    # single TileContext: phase barriers replaced by semaphore-gated
    # AllGathers; d-tables live in SBUF for the whole program
    with (
        tile.TileContext(nc) as tc,
        tc.tile_pool(name="glob", bufs=1) as gp,
    ):
        dtab1 = gp.tile([128, NT, 8], bf16)
        dtab2 = gp.tile([128, NT, 8], bf16)
        # ---------- phase 1: t1 = xT.T @ W1a ; pack tables ----------
        with (
            tc.tile_pool(name="w", bufs=1) as wp,
            tc.tile_pool(name="a", bufs=3) as ap,
            tc.tile_pool(name="ps", bufs=2, space="PSUM") as pp,
        ):
            w1_t = wp.tile([128, 2, RW1], f16)
            nc.sync.dma_start(w1_t[:], w1[:, :].rearrange("(k p) c -> p k c", p=128))
            for t in range(NT):
                xt8 = ap.tile([128, 2, 128], f8, tag="xt8")
                nc.sync.dma_start(
                    xt8[:],
                    xT[:, t * 128:(t + 1) * 128].rearrange("(k p) c -> p k c", p=128))
                xt = ap.tile([128, 2, 128], f16, tag="xt")
                nc.vector.tensor_copy(xt[:], xt8[:])
                acc = pp.tile([128, RW1], f32, tag="acc")
                nc.tensor.matmul(out=acc[:], lhsT=xt[:, 0, :],
                                 rhs=w1_t[:, 0, :], start=True, stop=False)
                nc.tensor.matmul(out=acc[:], lhsT=xt[:, 1, :],
                                 rhs=w1_t[:, 1, :], start=False, stop=True)
                # pack row: h bf16, s hi/lo
                row = ap.tile([128, RW1], bf16, tag="row")
                nc.vector.tensor_copy(row[:, 0:256], acc[:, 0:256])
                s_hi32 = ap.tile([128, 4], f32, tag="shi32")
                nc.vector.tensor_copy(row[:, 256:260], acc[:, 256:260])
                nc.vector.tensor_copy(s_hi32[:], row[:, 256:260])
                s_lo = ap.tile([128, 4], f32, tag="slo")
                nc.vector.tensor_tensor(out=s_lo[:], in0=acc[:, 256:260],
                                        in1=s_hi32[:], op=ALU.subtract)
                nc.vector.tensor_copy(row[:, 260:264], s_lo[:])
                nc.sync.dma_start(
                    t1_loc[t * 128:(t + 1) * 128, :], row[:])
                # d1 hi/lo straight into the SBUF-resident table
                d_hi32 = ap.tile([128, 4], f32, tag="dhi32")
                nc.vector.tensor_copy(dtab1[:, t, 0:4], acc[:, 260:264])
                nc.vector.tensor_copy(d_hi32[:], dtab1[:, t, 0:4])
                d_lo = ap.tile([128, 4], f32, tag="dlo")
                nc.vector.tensor_tensor(out=d_lo[:], in0=acc[:, 260:264],
                                        in1=d_hi32[:], op=ALU.subtract)
                nc.vector.tensor_copy(dtab1[:, t, 4:8], d_lo[:])
        nc.gpsimd.collective_compute(
            "AllGather", mybir.AluOpType.bypass,
            replica_groups=[list(range(NC))],
            ins=[t1_loc[:, :].opt()], outs=[T1[:, :].opt()],
        )
        # ---------- L1 message passing -> h2 -> fused t2, pack T2 ----------
        message_pass(tc, T1, dtab1, RW1, 256, "a", l1_out)
        nc.gpsimd.collective_compute(
            "AllGather", mybir.AluOpType.bypass,
            replica_groups=[list(range(NC))],
            ins=[t2_loc[:, :].opt()], outs=[T2[:, :].opt()],
        )
        # ---------- L2 message passing -> log_softmax -> out ----------
        message_pass(tc, T2, dtab2, RW2, 128, "b", l2_out)

    return nc


def _split_sync_waits(nc, max_waits=1):
    import concourse.mybir as mybir
    ctr = [0]
    for f in nc.m.functions:
        for blk in f.blocks:
            new_list = []
            for ins in blk.instructions:
                si = ins.sync_info
                waits = list(si.on_wait) if si is not None and si.on_wait else []
                if len(waits) > max_waits:
                    keep = waits[:max_waits]
                    rest = waits[max_waits:]
                    for i in range(0, len(rest), max_waits):
                        ctr[0] += 1
                        nop = mybir.InstNoOp(
                            name=f"I-wsplit-{ctr[0]}", ins=[], outs=[],
                            engine=ins.engine)
                        nop.sync_info = mybir.SyncInfo(
                            on_wait=rest[i:i + max_waits], on_update=[])
                        new_list.append(nop)
                    ins.sync_info = mybir.SyncInfo(
                        on_wait=keep,
                        on_update=list(si.on_update) if si.on_update else [])
                new_list.append(ins)
            blk.instructions[:] = new_list


_CACHE = {}


def _get_runner():
    """Build (once) the jitted SPMD executor for the Bass program.

    Mirrors concourse.bass2jax.run_bass_via_pjrt's multi-core path, with
    three fixes: the jitted callable + on-device zero output buffers are
    cached (no per-call retrace/XLA-compile/zeros upload), outputs are
    fetched once (not once per core), and the JAX persistent compilation
    cache is enabled so fresh processes skip the walrus compile.
    """
    if "runner" in _CACHE:
        return _CACHE["runner"]
    import jax
    import jax.numpy as jnp
    from jax.sharding import Mesh, PartitionSpec
    try:
        from jax.experimental.shard_map import shard_map
    except ImportError:
        from jax import shard_map
    from concourse import mybir
    from concourse.bass2jax import (
        _bass_exec_p, install_neuronx_cc_hook, partition_id_tensor)

    try:
        jax.config.update("jax_compilation_cache_dir", "/tmp/jax_bass_cache")
        jax.config.update("jax_persistent_cache_min_compile_time_secs", 0)
        jax.config.update("jax_persistent_cache_min_entry_size_bytes", 0)
    except Exception:
        pass

    nc = _build_nc()
    _split_sync_waits(nc, 1)
    install_neuronx_cc_hook()
    assert nc.dbg_addr is None

    in_names = []
    out_names = []
    out_avals = []
    partition_name = (nc.partition_id_tensor.name
                      if nc.partition_id_tensor else None)
    for alloc in nc.m.functions[0].allocations:
        if not isinstance(alloc, mybir.MemoryLocationSet):
            continue
        name = alloc.memorylocations[0].name
        if alloc.kind == "ExternalInput":
            if name != partition_name:
                in_names.append(name)
        elif alloc.kind == "ExternalOutput":
            shape = tuple(alloc.tensor_shape)
            dtype = mybir.dt.np(alloc.dtype)
            out_names.append(name)
            out_avals.append(jax.core.ShapedArray(shape, dtype))
    n_params = len(in_names)
    full_in_names = list(in_names) + list(out_names)
    if partition_name is not None:
        full_in_names.append(partition_name)

    def _body(*args):
        operands = list(args)
        if partition_name is not None:
            operands.append(partition_id_tensor())
        outs = _bass_exec_p.bind(
            *operands,
            out_avals=tuple(out_avals),
            in_names=tuple(full_in_names),
            out_names=tuple(out_names),
            lowering_input_output_aliases=(),
            sim_require_finite=True,
            sim_require_nnan=True,
            nc=nc,
        )
        return tuple(outs)

    devices = jax.devices()[:NC]
    mesh = Mesh(np.asarray(devices), ("core",))
    n_total = n_params + len(out_names)
    sharded = jax.jit(
        shard_map(_body, mesh=mesh,
                  in_specs=(PartitionSpec("core"),) * n_total,
                  out_specs=(PartitionSpec("core"),) * len(out_names),
                  check_rep=False),
        keep_unused=True,
    )

    # zero output operands, materialized on device (never transferred)
    zfun = jax.jit(
        shard_map(
            lambda: tuple(jnp.zeros(a.shape, a.dtype) for a in out_avals),
            mesh=mesh, in_specs=(),
            out_specs=(PartitionSpec("core"),) * len(out_avals),
            check_rep=False))
    zeros = [z for z in zfun()]

    _CACHE["runner"] = (sharded, in_names, out_names, out_avals, zeros)
    return _CACHE["runner"]


def kernel(**inputs):
    import time as _time

    x = np.asarray(inputs["x"], np.float32)
    ei = np.asarray(inputs["edge_index"])
    W1a, W2a, idx_t, dl_t, dlT_t, xs, b1r, b2r = _host_prep(
        x, ei, inputs["W1"], inputs["att_src1"], inputs["att_dst1"],
        inputs["b1"], inputs["W2"], inputs["att_src2"], inputs["att_dst2"],
        inputs["b2"])

    sharded, in_names, out_names, out_avals, zeros = _get_runner()

    per_core = {
        "xT": xs, "idx": idx_t, "dl": dl_t, "dlT": dlT_t,
        "w1": np.broadcast_to(W1a, (NC,) + W1a.shape),
        "w2": np.broadcast_to(W2a, (NC,) + W2a.shape),
        "b1r": np.broadcast_to(b1r, (NC,) + b1r.shape),
        "b2r": np.broadcast_to(b2r, (NC,) + b2r.shape),
    }
    concat_in = [
        np.ascontiguousarray(per_core[name].reshape(
            NC * per_core[name].shape[1], *per_core[name].shape[2:]))
        for name in in_names
    ]

    t0 = _time.time()
    out_arrs = sharded(*concat_in, *zeros)
    res = {name: np.asarray(out_arrs[i]) for i, name in enumerate(out_names)}
    wall = _time.time() - t0
    kernel.last_wall_s = wall
    kernel.last_exec_ns = None

    q = res["out"].reshape(NC, NSHP, H * C2)
    o = q[:, :NSH].astype(np.float32) * (12.0 / 255.0) - 12.0
    kernel.last_concat_in = concat_in
    return np.ascontiguousarray(o).reshape(N, H * C2)


def measure_hw_exec_ns(iters=16):
    """Measure on-device execution time of the compiled SPMD program.

    Uploads the inputs once, then launches `iters` back-to-back executions
    (async dispatch pipelines them) and returns the amortized per-run wall
    time in ns. This approximates the neuron-profile NEFF execution time
    (upper bound: includes per-dispatch driver overhead).
    """
    import time as _time
    import jax
    from jax.sharding import Mesh, PartitionSpec, NamedSharding

    concat_in = kernel.last_concat_in
    sharded, in_names, out_names, out_avals, zeros = _get_runner()
    mesh = Mesh(np.asarray(jax.devices()[:NC]), ("core",))
    sh = NamedSharding(mesh, PartitionSpec("core"))
    dev_in = [jax.device_put(a, sh) for a in concat_in]
    jax.block_until_ready(dev_in)
    # warm (retrace for device-array args) + sanity
    out = sharded(*dev_in, *zeros)
    jax.block_until_ready(out)
    best = None
    for _ in range(5):
        t0 = _time.time()
        outs = [sharded(*dev_in, *zeros) for _ in range(iters)]
        jax.block_until_ready(outs)
        dt = (_time.time() - t0) / iters
        best = dt if best is None else min(best, dt)
    return int(best * 1e9)
